# revision 1
# baseline (speedup 1.0000x reference)
"""Trainium2 Bass kernel for nn_DiffusionDynamicInput.

Reference computation (per sample b):
    ctx  = wv_embs[b] + t_emb[b]                       (13, 1024)
    hid  = silu(ctx @ w1 + b1)                         (13, 512)
    wgen = (hid @ w2 + b2).reshape(13, 128, 9)         per-(band) 3x3 filters
    out[d,h,w] = sum_{n,dy,dx} wgen[n,d,(dy,dx)] * x[b,n,h+dy,w+dx]   (SAME pad)
    bias = (ctx @ wb + bb).sum(axis=0)                 (128,)
    out += bias[:, None, None]

Sharding: data-parallel over B=8 across the 8 NeuronCores (one sample per
core). Per core the dynamic conv runs as K=39 fp16 matmuls: partition
q = n*3 + dyi holds the full image of band n shifted by dy (rows stored
258 wide with zero pad columns, so the dx shift is a free-dim offset);
the three dx matmuls accumulate in one PSUM bank. x arrives host-cast
to fp16 and host-padded to 258-wide rows, so the shifted replicas are
three fully-contiguous DMA loads into a resident SBUF image
(132 KB/partition). The hypernetwork runs with fp16 operands (host-cast,
host-permuted weights) and fp32 PSUM. The per-sample bias and the
PSUM->SBUF eviction are fused; output DMAs alternate between the two
HWDGE rings (SP/ACT) since the 33.5 MB/core output write is the
bandwidth bottleneck.
"""

import numpy as np

import concourse.bacc as bacc
import concourse.mybir as mybir
import concourse.tile as tile
from concourse.bass_utils import run_bass_kernel_spmd
from concourse.masks import make_identity

F32 = mybir.dt.float32
F16 = mybir.dt.float16

NB = 13          # bands
HH = WW = 256    # image
DE = 1024        # embed dim
DO = 128         # out channels
NCORES = 8

WPAD = WW + 2    # 258: row layout with a zero column at each end
GRP = 8          # psum banks in flight
OSTROWS = 8      # output rows per staging tile / output DMA (1 MB DMAs)


def _build_bass(repeat: int = 1, ablate: str = ""):
    # Bacc (not plain Bass): its finalize() runs generate_event_semaphores,
    # which splits multi-sem waits that TRN2 instruction structs can't hold.
    # repeat > 1 re-emits the main conv loop (benchmarking: slope between
    # repeat counts isolates device time from dispatch overhead).
    ab = set(ablate.split(",")) if ablate else set()
    nc = bacc.Bacc(target_bir_lowering=False, debug=False)

    # x is host-cast to fp16 and host-padded to 258-wide rows (zero col at
    # each end), so the im2col DMAs are fully contiguous per partition
    x_ext = nc.declare_dram_parameter("x", [NB, HH, WPAD], F16, isOutput=False)
    t_ext = nc.declare_dram_parameter("t_emb", [DE], F32, isOutput=False)
    wv_ext = nc.declare_dram_parameter("wv", [NB, DE], F32, isOutput=False)
    # w1/w2p/wb are host-cast to fp16; w2p/b2p host-permuted so generated
    # filter column c' = p*128 + d
    # w1p[p, k, m*128+s] = w1[k*128+p, m*128+s]; similarly w2p along k;
    # wbp[p, k, d] = wb[k*128+p, d]  (one contiguous DMA per weight)
    w1_ext = nc.declare_dram_parameter("w1p", [128, 8, 4 * DO], F16, isOutput=False)
    b1_ext = nc.declare_dram_parameter("b1", [4 * DO], F32, isOutput=False)
    w2p_ext = nc.declare_dram_parameter("w2pp", [128, 4, DO * 9], F16, isOutput=False)
    b2p_ext = nc.declare_dram_parameter("b2p", [DO * 9], F16, isOutput=False)
    wb_ext = nc.declare_dram_parameter("wbp", [128, 8, DO], F16, isOutput=False)
    bb_ext = nc.declare_dram_parameter("bb", [DO], F32, isOutput=False)
    out_ext = nc.declare_dram_parameter("out", [DO, HH, WW], F32, isOutput=True)

    with tile.TileContext(nc) as tc:
        with (
            tc.tile_pool(name="const", bufs=1) as const_pool,
            tc.tile_pool(name="resident", bufs=1) as res_pool,
            tc.tile_pool(name="hyp", bufs=1) as hyp_pool,
        ):
            # ---------------- hypernetwork (fp16 in / fp32 psum) ------------
            ident = const_pool.tile([128, 128], F32)
            make_identity(nc, ident[:])

            tT = hyp_pool.tile([128, 8], F32)   # t_emb[k*128+p] -> [p, k]
            nc.sync.dma_start(tT[:], t_ext.ap().rearrange("(k p) -> p k", p=128))
            b1T = hyp_pool.tile([128, 4], F32)
            nc.sync.dma_start(b1T[:], b1_ext.ap().rearrange("(m p) -> p m", p=128))
            bbT = hyp_pool.tile([128, 1], F32)
            nc.sync.dma_start(bbT[:], bb_ext.ap().rearrange("(p o) -> p o", o=1))
            b2pT = hyp_pool.tile([1, DO * 9], F16)
            nc.sync.dma_start(b2pT[:], b2p_ext.ap().rearrange("(o c) -> o c", o=1))
            ones1 = const_pool.tile([1, NB], F16)
            nc.vector.memset(ones1[:], 1.0)

            wv_t = hyp_pool.tile([NB, DE], F32)
            nc.sync.dma_start(wv_t[:], wv_ext.ap())

            w1p_t = hyp_pool.tile([128, 8, 4 * DO], F16)
            nc.sync.dma_start(w1p_t[:], w1_ext.ap())
            w2p_t = hyp_pool.tile([128, 4, DO * 9], F16)
            nc.sync.dma_start(w2p_t[:], w2p_ext.ap())
            wbp_t = hyp_pool.tile([128, 8, DO], F16)
            nc.sync.dma_start(wbp_t[:], wb_ext.ap())

            # ctxT[e, k, n] = wv[n, k*128+e] + t[k*128+e]   (fp16)
            ctxT = hyp_pool.tile([128, 8, NB], F16)
            with tc.tile_pool(name="tp_psum", bufs=2, space="PSUM") as tp_psum:
                # warm-up op: absorbs the identity-producer (Pool) semaphore
                # into the PE engine clock so later transposes carry a single
                # wait (the fused LDW struct has one wait slot).
                ps_warm = tp_psum.tile([1, 1], F32, tag="warm", bufs=1)
                nc.tensor.transpose(ps_warm[:], ident[:1, :1], ident[:1, :1])
                for k in range(8):
                    ps = tp_psum.tile([128, NB], F32, tag="tp")
                    nc.tensor.transpose(
                        ps[:], wv_t[:, k * 128:(k + 1) * 128], ident[:NB, :NB]
                    )
                    nc.vector.tensor_scalar_add(ctxT[:, k, :], ps[:], tT[:, k:k + 1])

                # sT[e, k] = sum_n ctxT[e, k, n]   (fp16 for the wb matmul)
                sT32 = hyp_pool.tile([128, 8, 1], F32)
                nc.vector.reduce_sum(sT32[:], ctxT[:], axis=mybir.AxisListType.X)
                sT = hyp_pool.tile([128, 8, 1], F16)
                nc.vector.tensor_copy(sT[:], sT32[:])

                # hidT[s, m, n] = silu(sum_e w1[e, m*128+s] * ctxT[e, n] + b1)
                hidT = hyp_pool.tile([128, 4, NB], F16)
                for m in range(4):
                    ps = tp_psum.tile([128, NB], F32, tag="hid")
                    for k in range(8):
                        nc.tensor.matmul(
                            ps[:], w1p_t[:, k, m * 128:(m + 1) * 128],
                            ctxT[:, k, :], start=(k == 0), stop=(k == 7)
                        )
                    nc.scalar.activation(
                        hidT[:, m, :], ps[:],
                        mybir.ActivationFunctionType.Silu, bias=b1T[:, m:m + 1],
                    )

                # wgen16[n, p*128+d] = hid @ w2p + b2p   (fp16)
                wgen16 = hyp_pool.tile([NB, DO * 9], F16)
                for j in range(3):  # 1152 = 3 * 384
                    ps = tp_psum.tile([NB, 384], F32, tag="wgen")
                    for k in range(4):
                        nc.tensor.matmul(
                            ps[:], hidT[:, k, :],
                            w2p_t[:, k, j * 384:(j + 1) * 384],
                            start=(k == 0), stop=False,
                        )
                    nc.tensor.matmul(
                        ps[:], ones1[:], b2pT[:, j * 384:(j + 1) * 384],
                        start=False, stop=True,
                    )
                    nc.vector.tensor_copy(wgen16[:, j * 384:(j + 1) * 384], ps[:])

                # bias[d] = sum_e s[e] * wb[e, d] + 13 * bb[d]
                bb13 = hyp_pool.tile([128, 1], F32)
                nc.vector.tensor_scalar_mul(bb13[:], bbT[:], float(NB))
                ps_b = tp_psum.tile([128, 1], F32, tag="bias", bufs=1)
                for k in range(8):
                    nc.tensor.matmul(
                        ps_b[:], wbp_t[:, k, :], sT[:, k, :],
                        start=(k == 0), stop=(k == 7)
                    )
                bias_sb = hyp_pool.tile([128, 1], F32)
                nc.scalar.activation(
                    bias_sb[:], ps_b[:],
                    mybir.ActivationFunctionType.Identity, bias=bb13[:],
                )

            # lhsT[dx][n*3+dyi, d] = wgen16[n, (dyi*3+dxi)*128 + d]
            # NOTE: only dim 0 of an SBUF AP crosses partitions, so one DMA
            # per (dx, dy): partition stride 3, offset dyi.
            lhsT = [
                hyp_pool.tile([3 * NB, DO], F16, tag=f"lhsT{i}", name=f"lhsT{i}")
                for i in range(3)
            ]
            wgen16_4d = wgen16[:].rearrange("n (dy dx d) -> n dy dx d", dy=3, dx=3)
            for dxi in range(3):
                lhsT_g = lhsT[dxi][:].rearrange("(n dy) d -> n dy d", dy=3)
                for dyi in range(3):
                    nc.sync.dma_start(
                        lhsT_g[:, dyi, :],
                        wgen16_4d[:, dyi, dxi, :],
                    )

            # ------- phase 0: build the dy-shifted fp16 image in SBUF -------
            # x39[n*3+dyi, r, 1+c] = x[n, r+dy, c]   (zeros at pads / edges)
            x39 = res_pool.tile([3 * NB, HH, WPAD], F16)
            # rows no DMA writes (image edge): zero across all partitions
            # first; the in-range dy groups' DMAs overwrite. Pad columns come
            # from the host-padded source rows.
            nc.gpsimd.memset(x39[:, 0:1, :], 0.0)
            nc.gpsimd.memset(x39[:, HH - 1:HH, :], 0.0)
            x39_g = x39[:].rearrange("(n dy) r w -> n dy r w", dy=3)
            for dyi, dy in enumerate((-1, 0, 1)):
                lo = max(0, -dy)
                hi = min(HH, HH - dy)
                nc.sync.dma_start(
                    x39_g[:, dyi, lo:hi, :],
                    x_ext.ap()[:, lo + dy:hi + dy, :],
                )

            # ---------------- main loop: dynamic conv -----------------------
            NPAIRS = HH // 2                    # 128 two-row pairs
            with (
                tc.tile_pool(name="ostage", bufs=4) as ostage_pool,
                tc.tile_pool(name="cpsum", bufs=GRP, space="PSUM") as cpsum_pool,
            ):
                for _rep in range(repeat):
                    for grp in range(NPAIRS // GRP):
                        psums = [
                            cpsum_pool.tile(
                                [DO, 2, WW], F32, tag="cps", name=f"cps{g}"
                            )
                            for g in range(GRP)
                        ]
                        # dx order (0, -1, +1): the dx=0 matmul reads no pad
                        # columns, keeping its wait count minimal.
                        dx_steps = (1,) if "mm1" in ab else (1, 0, 2)
                        for step, dxi in enumerate(dx_steps):
                            for g in range(GRP):
                                r0 = (grp * GRP + g) * 2
                                nc.tensor.matmul(
                                    psums[g][:],
                                    lhsT[dxi][:],
                                    x39[:, r0:r0 + 2, dxi:dxi + WW],
                                    start=(step == 0),
                                    stop=(step == len(dx_steps) - 1),
                                )
                        for ost_i in range(GRP * 2 // OSTROWS):
                            y0 = grp * GRP * 2 + ost_i * OSTROWS
                            ost = ostage_pool.tile([DO, OSTROWS, WW], F32, tag="ost")
                            for e in range(OSTROWS // 2):
                                g = ost_i * (OSTROWS // 2) + e
                                if g % 2 == 0:
                                    nc.scalar.activation(
                                        ost[:, 2 * e:2 * e + 2, :], psums[g][:],
                                        mybir.ActivationFunctionType.Identity,
                                        bias=bias_sb[:],
                                    )
                                else:
                                    nc.vector.tensor_scalar_add(
                                        ost[:, 2 * e:2 * e + 2, :], psums[g][:],
                                        bias_sb[:],
                                    )
                            # rotate output DMAs across SP ring, ACT ring,
                            # and the gpsimd SWDGE path
                            rot = (2 * grp + ost_i) % 3
                            dma_eng = (nc.sync, nc.scalar, nc.gpsimd)[rot]
                            if "outslim" in ab:
                                dma_eng.dma_start(
                                    out_ext.ap()[:, y0:y0 + OSTROWS, 0:16],
                                    ost[:, :, 0:16],
                                )
                            else:
                                dma_eng.dma_start(
                                    out_ext.ap()[:, y0:y0 + OSTROWS, :], ost[:]
                                )
    if not nc.is_finalized():
        nc.finalize()
    return nc


_NC_CACHE = None


def _get_bass():
    global _NC_CACHE
    if _NC_CACHE is None:
        _NC_CACHE = _build_bass()
    return _NC_CACHE


def _prep_in_maps(inputs):
    x16 = np.asarray(inputs["x"], dtype=np.float32).astype(np.float16)
    x = np.zeros((x16.shape[0], NB, HH, WPAD), np.float16)
    x[:, :, :, 1:WW + 1] = x16
    t_emb = np.ascontiguousarray(np.asarray(inputs["t_emb"], dtype=np.float32))
    wv = np.ascontiguousarray(np.asarray(inputs["wv_embs"], dtype=np.float32))
    w1 = np.asarray(inputs["w1"], dtype=np.float32)
    b1 = np.ascontiguousarray(np.asarray(inputs["b1"], dtype=np.float32))
    w2 = np.asarray(inputs["w2"], dtype=np.float32)
    b2 = np.asarray(inputs["b2"], dtype=np.float32)
    wb = np.asarray(inputs["wb"], dtype=np.float32)
    bb = np.ascontiguousarray(np.asarray(inputs["bb"], dtype=np.float32))

    # permute filter columns: c = d*9 + p  ->  c' = p*128 + d; cast to fp16
    w2p = w2.reshape(4 * DO, DO, 9).transpose(0, 2, 1).reshape(4 * DO, DO * 9)
    w2pp = np.ascontiguousarray(
        w2p.reshape(4, 128, DO * 9).transpose(1, 0, 2)
    ).astype(np.float16)
    b2p = np.ascontiguousarray(b2.reshape(DO, 9).T.reshape(DO * 9)).astype(np.float16)
    w1p = np.ascontiguousarray(
        w1.reshape(8, 128, 4 * DO).transpose(1, 0, 2)
    ).astype(np.float16)
    wbp = np.ascontiguousarray(
        wb.reshape(8, 128, DO).transpose(1, 0, 2)
    ).astype(np.float16)

    return [
        {
            "x": x[b], "t_emb": t_emb[b], "wv": wv[b],
            "w1p": w1p, "b1": b1, "w2pp": w2pp, "b2p": b2p,
            "wbp": wbp, "bb": bb,
        }
        for b in range(NCORES)
    ]


def kernel(**inputs) -> np.ndarray:
    nc = _get_bass()
    in_maps = _prep_in_maps(inputs)
    res = run_bass_kernel_spmd(nc, in_maps, list(range(NCORES)))
    return np.stack([res.results[b]["out"] for b in range(NCORES)], axis=0)


if __name__ == "__main__":
    rng = np.random.default_rng(0)
    demo = {
        "x": rng.standard_normal((NCORES, NB, HH, WW), dtype=np.float32),
        "t_emb": rng.standard_normal((NCORES, DE), dtype=np.float32),
        "wv_embs": rng.standard_normal((NCORES, NB, DE), dtype=np.float32),
        "w1": rng.standard_normal((DE, 4 * DO), dtype=np.float32) * 0.02,
        "b1": np.zeros(4 * DO, np.float32),
        "w2": rng.standard_normal((4 * DO, DO * 9), dtype=np.float32) * 0.02,
        "b2": np.zeros(DO * 9, np.float32),
        "wb": rng.standard_normal((DE, DO), dtype=np.float32) * 0.02,
        "bb": np.zeros(DO, np.float32),
    }
    out = kernel(**demo)
    print("out", out.shape, out.dtype, float(np.abs(out).mean()))



# revision 4
# speedup vs baseline: 1.1017x; 1.1017x over previous
"""Trainium2 Bass kernel for nn_DiffusionDynamicInput.

Reference computation (per sample b):
    ctx  = wv_embs[b] + t_emb[b]                       (13, 1024)
    hid  = silu(ctx @ w1 + b1)                         (13, 512)
    wgen = (hid @ w2 + b2).reshape(13, 128, 9)         per-(band) 3x3 filters
    out[d,h,w] = sum_{n,dy,dx} wgen[n,d,(dy,dx)] * x[b,n,h+dy,w+dx]   (SAME pad)
    bias = (ctx @ wb + bb).sum(axis=0)                 (128,)
    out += bias[:, None, None]

Sharding: data-parallel over B=8 across the 8 NeuronCores (one sample per
core). Per core the dynamic conv runs as K=39 fp16 matmuls: partition
q = n*3 + dyi holds the full image of band n shifted by dy (rows stored
258 wide with zero pad columns, so the dx shift is a free-dim offset);
the three dx matmuls accumulate in one PSUM bank. x arrives host-cast
to fp16 and host-padded to 258-wide rows, so the shifted replicas are
three fully-contiguous DMA loads into a resident SBUF image
(132 KB/partition). The hypernetwork runs with fp16 operands (host-cast,
host-permuted weights) and fp32 PSUM. The per-sample bias and the
PSUM->SBUF eviction are fused; output DMAs alternate between the two
HWDGE rings (SP/ACT) since the 33.5 MB/core output write is the
bandwidth bottleneck.
"""

import numpy as np

import concourse.bacc as bacc
import concourse.mybir as mybir
import concourse.tile as tile
from concourse.bass_utils import run_bass_kernel_spmd
from concourse.masks import make_identity

F32 = mybir.dt.float32
F16 = mybir.dt.float16

NB = 13          # bands
HH = WW = 256    # image
DE = 1024        # embed dim
DO = 128         # out channels
NCORES = 8

WPAD = WW + 2    # 258: row layout with a zero column at each end
GRP = 8          # psum banks in flight
OSTROWS = 8      # output rows per staging tile / output DMA (1 MB DMAs)


def _build_bass(repeat: int = 1, ablate: str = ""):
    # Bacc (not plain Bass): its finalize() runs generate_event_semaphores,
    # which splits multi-sem waits that TRN2 instruction structs can't hold.
    # repeat > 1 re-emits the main conv loop (benchmarking: slope between
    # repeat counts isolates device time from dispatch overhead).
    ab = set(ablate.split(",")) if ablate else set()
    nc = bacc.Bacc(target_bir_lowering=False, debug=False)

    # x is host-cast to fp16 and host-padded to 258-wide rows (zero col at
    # each end), so the im2col DMAs are fully contiguous per partition
    x_ext = nc.declare_dram_parameter("x", [NB, HH, WPAD], F16, isOutput=False)
    t_ext = nc.declare_dram_parameter("t_emb", [DE], F32, isOutput=False)
    wv_ext = nc.declare_dram_parameter("wv", [NB, DE], F32, isOutput=False)
    # w1/w2p/wb are host-cast to fp16; w2p/b2p host-permuted so generated
    # filter column c' = p*128 + d
    # w1p[p, k, m*128+s] = w1[k*128+p, m*128+s]; similarly w2p along k;
    # wbp[p, k, d] = wb[k*128+p, d]  (one contiguous DMA per weight)
    w1_ext = nc.declare_dram_parameter("w1p", [128, 8, 4 * DO], F16, isOutput=False)
    b1_ext = nc.declare_dram_parameter("b1", [4 * DO], F32, isOutput=False)
    w2p_ext = nc.declare_dram_parameter("w2pp", [128, 4, DO * 9], F16, isOutput=False)
    b2p_ext = nc.declare_dram_parameter("b2p", [DO * 9], F16, isOutput=False)
    wb_ext = nc.declare_dram_parameter("wbp", [128, 8, DO], F16, isOutput=False)
    bb_ext = nc.declare_dram_parameter("bb", [DO], F32, isOutput=False)
    # fp16 output (33.5 MB fp32 -> 16.8 MB): the output write is the dominant
    # DMA cost; host upcasts to fp32 after gather (rel tol is 2e-2).
    out_ext = nc.declare_dram_parameter("out", [DO, HH, WW], F16, isOutput=True)

    with tile.TileContext(nc) as tc:
        with (
            tc.tile_pool(name="const", bufs=1) as const_pool,
            tc.tile_pool(name="resident", bufs=1) as res_pool,
            tc.tile_pool(name="hyp", bufs=1) as hyp_pool,
        ):
            # ---------------- hypernetwork (fp16 in / fp32 psum) ------------
            ident = const_pool.tile([128, 128], F32)
            make_identity(nc, ident[:])

            tT = hyp_pool.tile([128, 8], F32)   # t_emb[k*128+p] -> [p, k]
            nc.sync.dma_start(tT[:], t_ext.ap().rearrange("(k p) -> p k", p=128))
            b1T = hyp_pool.tile([128, 4], F32)
            nc.sync.dma_start(b1T[:], b1_ext.ap().rearrange("(m p) -> p m", p=128))
            bbT = hyp_pool.tile([128, 1], F32)
            nc.sync.dma_start(bbT[:], bb_ext.ap().rearrange("(p o) -> p o", o=1))
            b2pT = hyp_pool.tile([1, DO * 9], F16)
            nc.sync.dma_start(b2pT[:], b2p_ext.ap().rearrange("(o c) -> o c", o=1))
            ones1 = const_pool.tile([1, NB], F16)
            nc.vector.memset(ones1[:], 1.0)

            wv_t = hyp_pool.tile([NB, DE], F32)
            nc.sync.dma_start(wv_t[:], wv_ext.ap())

            w1p_t = hyp_pool.tile([128, 8, 4 * DO], F16)
            nc.sync.dma_start(w1p_t[:], w1_ext.ap())
            w2p_t = hyp_pool.tile([128, 4, DO * 9], F16)
            nc.sync.dma_start(w2p_t[:], w2p_ext.ap())
            wbp_t = hyp_pool.tile([128, 8, DO], F16)
            nc.sync.dma_start(wbp_t[:], wb_ext.ap())

            # ctxT[e, k, n] = wv[n, k*128+e] + t[k*128+e]   (fp16)
            ctxT = hyp_pool.tile([128, 8, NB], F16)
            with tc.tile_pool(name="tp_psum", bufs=2, space="PSUM") as tp_psum:
                # warm-up op: absorbs the identity-producer (Pool) semaphore
                # into the PE engine clock so later transposes carry a single
                # wait (the fused LDW struct has one wait slot).
                ps_warm = tp_psum.tile([1, 1], F32, tag="warm", bufs=1)
                nc.tensor.transpose(ps_warm[:], ident[:1, :1], ident[:1, :1])
                for k in range(8):
                    ps = tp_psum.tile([128, NB], F32, tag="tp")
                    nc.tensor.transpose(
                        ps[:], wv_t[:, k * 128:(k + 1) * 128], ident[:NB, :NB]
                    )
                    nc.vector.tensor_scalar_add(ctxT[:, k, :], ps[:], tT[:, k:k + 1])

                # sT[e, k] = sum_n ctxT[e, k, n]   (fp16 for the wb matmul)
                sT32 = hyp_pool.tile([128, 8, 1], F32)
                nc.vector.reduce_sum(sT32[:], ctxT[:], axis=mybir.AxisListType.X)
                sT = hyp_pool.tile([128, 8, 1], F16)
                nc.vector.tensor_copy(sT[:], sT32[:])

                # hidT[s, m, n] = silu(sum_e w1[e, m*128+s] * ctxT[e, n] + b1)
                hidT = hyp_pool.tile([128, 4, NB], F16)
                for m in range(4):
                    ps = tp_psum.tile([128, NB], F32, tag="hid")
                    for k in range(8):
                        nc.tensor.matmul(
                            ps[:], w1p_t[:, k, m * 128:(m + 1) * 128],
                            ctxT[:, k, :], start=(k == 0), stop=(k == 7)
                        )
                    nc.scalar.activation(
                        hidT[:, m, :], ps[:],
                        mybir.ActivationFunctionType.Silu, bias=b1T[:, m:m + 1],
                    )

                # wgen16[n, p*128+d] = hid @ w2p + b2p   (fp16)
                wgen16 = hyp_pool.tile([NB, DO * 9], F16)
                for j in range(3):  # 1152 = 3 * 384
                    ps = tp_psum.tile([NB, 384], F32, tag="wgen")
                    for k in range(4):
                        nc.tensor.matmul(
                            ps[:], hidT[:, k, :],
                            w2p_t[:, k, j * 384:(j + 1) * 384],
                            start=(k == 0), stop=False,
                        )
                    nc.tensor.matmul(
                        ps[:], ones1[:], b2pT[:, j * 384:(j + 1) * 384],
                        start=False, stop=True,
                    )
                    nc.vector.tensor_copy(wgen16[:, j * 384:(j + 1) * 384], ps[:])

                # bias[d] = sum_e s[e] * wb[e, d] + 13 * bb[d]
                bb13 = hyp_pool.tile([128, 1], F32)
                nc.vector.tensor_scalar_mul(bb13[:], bbT[:], float(NB))
                ps_b = tp_psum.tile([128, 1], F32, tag="bias", bufs=1)
                for k in range(8):
                    nc.tensor.matmul(
                        ps_b[:], wbp_t[:, k, :], sT[:, k, :],
                        start=(k == 0), stop=(k == 7)
                    )
                bias_sb = hyp_pool.tile([128, 1], F32)
                nc.scalar.activation(
                    bias_sb[:], ps_b[:],
                    mybir.ActivationFunctionType.Identity, bias=bb13[:],
                )

            # lhsT[dx][n*3+dyi, d] = wgen16[n, (dyi*3+dxi)*128 + d]
            # NOTE: only dim 0 of an SBUF AP crosses partitions, so one DMA
            # per (dx, dy): partition stride 3, offset dyi.
            lhsT = [
                hyp_pool.tile([3 * NB, DO], F16, tag=f"lhsT{i}", name=f"lhsT{i}")
                for i in range(3)
            ]
            wgen16_4d = wgen16[:].rearrange("n (dy dx d) -> n dy dx d", dy=3, dx=3)
            for dxi in range(3):
                lhsT_g = lhsT[dxi][:].rearrange("(n dy) d -> n dy d", dy=3)
                for dyi in range(3):
                    nc.sync.dma_start(
                        lhsT_g[:, dyi, :],
                        wgen16_4d[:, dyi, dxi, :],
                    )

            # ------- phase 0: build the dy-shifted fp16 image in SBUF -------
            # x39[n*3+dyi, r, 1+c] = x[n, r+dy, c]   (zeros at pads / edges)
            x39 = res_pool.tile([3 * NB, HH, WPAD], F16)
            # rows no DMA writes (image edge): zero across all partitions
            # first; the in-range dy groups' DMAs overwrite. Pad columns come
            # from the host-padded source rows.
            nc.gpsimd.memset(x39[:, 0:1, :], 0.0)
            nc.gpsimd.memset(x39[:, HH - 1:HH, :], 0.0)
            x39_g = x39[:].rearrange("(n dy) r w -> n dy r w", dy=3)
            for dyi, dy in enumerate((-1, 0, 1)):
                lo = max(0, -dy)
                hi = min(HH, HH - dy)
                nc.sync.dma_start(
                    x39_g[:, dyi, lo:hi, :],
                    x_ext.ap()[:, lo + dy:hi + dy, :],
                )

            # ---------------- main loop: dynamic conv -----------------------
            NPAIRS = HH // 2                    # 128 two-row pairs
            with (
                tc.tile_pool(name="ostage", bufs=4) as ostage_pool,
                tc.tile_pool(name="cpsum", bufs=GRP, space="PSUM") as cpsum_pool,
            ):
                for _rep in range(repeat):
                    for grp in range(NPAIRS // GRP):
                        psums = [
                            cpsum_pool.tile(
                                [DO, 2, WW], F32, tag="cps", name=f"cps{g}"
                            )
                            for g in range(GRP)
                        ]
                        # dx order (0, -1, +1): the dx=0 matmul reads no pad
                        # columns, keeping its wait count minimal.
                        dx_steps = (1,) if "mm1" in ab else (1, 0, 2)
                        for step, dxi in enumerate(dx_steps):
                            for g in range(GRP):
                                r0 = (grp * GRP + g) * 2
                                nc.tensor.matmul(
                                    psums[g][:],
                                    lhsT[dxi][:],
                                    x39[:, r0:r0 + 2, dxi:dxi + WW],
                                    start=(step == 0),
                                    stop=(step == len(dx_steps) - 1),
                                )
                        for ost_i in range(GRP * 2 // OSTROWS):
                            y0 = grp * GRP * 2 + ost_i * OSTROWS
                            ost = ostage_pool.tile([DO, OSTROWS, WW], F16, tag="ost")
                            for e in range(OSTROWS // 2):
                                g = ost_i * (OSTROWS // 2) + e
                                if g % 2 == 0:
                                    nc.scalar.activation(
                                        ost[:, 2 * e:2 * e + 2, :], psums[g][:],
                                        mybir.ActivationFunctionType.Identity,
                                        bias=bias_sb[:],
                                    )
                                else:
                                    nc.vector.tensor_scalar_add(
                                        ost[:, 2 * e:2 * e + 2, :], psums[g][:],
                                        bias_sb[:],
                                    )
                            # rotate output DMAs across SP ring, ACT ring,
                            # and the gpsimd SWDGE path
                            rot = (2 * grp + ost_i) % 3
                            dma_eng = (nc.sync, nc.scalar, nc.gpsimd)[rot]
                            if "outslim" in ab:
                                dma_eng.dma_start(
                                    out_ext.ap()[:, y0:y0 + OSTROWS, 0:16],
                                    ost[:, :, 0:16],
                                )
                            else:
                                dma_eng.dma_start(
                                    out_ext.ap()[:, y0:y0 + OSTROWS, :], ost[:]
                                )
    if not nc.is_finalized():
        nc.finalize()
    return nc


_NC_CACHE = None


def _get_bass():
    global _NC_CACHE
    if _NC_CACHE is None:
        _NC_CACHE = _build_bass()
    return _NC_CACHE


def _prep_in_maps(inputs):
    x16 = np.asarray(inputs["x"], dtype=np.float32).astype(np.float16)
    x = np.zeros((x16.shape[0], NB, HH, WPAD), np.float16)
    x[:, :, :, 1:WW + 1] = x16
    t_emb = np.ascontiguousarray(np.asarray(inputs["t_emb"], dtype=np.float32))
    wv = np.ascontiguousarray(np.asarray(inputs["wv_embs"], dtype=np.float32))
    w1 = np.asarray(inputs["w1"], dtype=np.float32)
    b1 = np.ascontiguousarray(np.asarray(inputs["b1"], dtype=np.float32))
    w2 = np.asarray(inputs["w2"], dtype=np.float32)
    b2 = np.asarray(inputs["b2"], dtype=np.float32)
    wb = np.asarray(inputs["wb"], dtype=np.float32)
    bb = np.ascontiguousarray(np.asarray(inputs["bb"], dtype=np.float32))

    # permute filter columns: c = d*9 + p  ->  c' = p*128 + d; cast to fp16
    w2p = w2.reshape(4 * DO, DO, 9).transpose(0, 2, 1).reshape(4 * DO, DO * 9)
    w2pp = np.ascontiguousarray(
        w2p.reshape(4, 128, DO * 9).transpose(1, 0, 2)
    ).astype(np.float16)
    b2p = np.ascontiguousarray(b2.reshape(DO, 9).T.reshape(DO * 9)).astype(np.float16)
    w1p = np.ascontiguousarray(
        w1.reshape(8, 128, 4 * DO).transpose(1, 0, 2)
    ).astype(np.float16)
    wbp = np.ascontiguousarray(
        wb.reshape(8, 128, DO).transpose(1, 0, 2)
    ).astype(np.float16)

    return [
        {
            "x": x[b], "t_emb": t_emb[b], "wv": wv[b],
            "w1p": w1p, "b1": b1, "w2pp": w2pp, "b2p": b2p,
            "wbp": wbp, "bb": bb,
        }
        for b in range(NCORES)
    ]


def kernel(**inputs) -> np.ndarray:
    nc = _get_bass()
    in_maps = _prep_in_maps(inputs)
    res = run_bass_kernel_spmd(nc, in_maps, list(range(NCORES)))
    return np.stack(
        [res.results[b]["out"] for b in range(NCORES)], axis=0
    ).astype(np.float32)


if __name__ == "__main__":
    rng = np.random.default_rng(0)
    demo = {
        "x": rng.standard_normal((NCORES, NB, HH, WW), dtype=np.float32),
        "t_emb": rng.standard_normal((NCORES, DE), dtype=np.float32),
        "wv_embs": rng.standard_normal((NCORES, NB, DE), dtype=np.float32),
        "w1": rng.standard_normal((DE, 4 * DO), dtype=np.float32) * 0.02,
        "b1": np.zeros(4 * DO, np.float32),
        "w2": rng.standard_normal((4 * DO, DO * 9), dtype=np.float32) * 0.02,
        "b2": np.zeros(DO * 9, np.float32),
        "wb": rng.standard_normal((DE, DO), dtype=np.float32) * 0.02,
        "bb": np.zeros(DO, np.float32),
    }
    out = kernel(**demo)
    print("out", out.shape, out.dtype, float(np.abs(out).mean()))



# revision 25
# speedup vs baseline: 1.3935x; 1.2649x over previous
"""Trainium2 Bass kernel for nn_DiffusionDynamicInput.

Reference computation (per sample b):
    ctx  = wv_embs[b] + t_emb[b]                       (13, 1024)
    hid  = silu(ctx @ w1 + b1)                         (13, 512)
    wgen = (hid @ w2 + b2).reshape(13, 128, 9)         per-(band) 3x3 filters
    out[d,h,w] = sum_{n,dy,dx} wgen[n,d,(dy,dx)] * x[b,n,h+dy,w+dx]   (SAME)
    bias = (ctx @ wb + bb).sum(axis=0)                 (128,)
    out += bias[:, None, None]

Sharding: data-parallel over B=8 across the 8 NeuronCores (one sample per
core).

Dynamic conv: an fp16 SBUF tile x118 [118, 256, 258] holds
  partitions  0..38  dy-shifted planes of the image (dy-major, host-baked)
  partition   39     all-ones (the generated bias rides the matmuls as an
                     extra contraction row against this partition)
  partitions 40..117 dx-shifted replicas of the planes, rows >= RA only
Rows < RA run "mode A": three PSUM-accumulated matmuls (contraction
39/39/40) with the dx shift as a free-dim column offset. Rows >= RA run
"mode B": one matmul with contraction 118 over (dx, dy, n) + bias. RA
balances PE column-streaming against the ~360 GB/s DMA budget (the fp16
output write dominates). Mode-B pairs are interleaved among mode-A pairs
(pattern A,A,B) so PSUM-eviction load stays smooth.

PSUM eviction is a plain fp32->fp16 copy alternating DVE/ACT; output DMAs
issue only from the SP/Pool queues so no sequencer mixes evictions with
blocking DMA waits. Output is written fp16 (rel tol 2e-2 vs ~4e-4
achieved) and upcast on host. Hypernetwork weights arrive host-permuted
(w1 m-major, w2 dx-major) so the dx=1 filter block - which gates the conv
start - is generated as early as possible.
"""

import numpy as np

import concourse.bacc as bacc
import concourse.mybir as mybir
import concourse.tile as tile
from concourse.bass_utils import run_bass_kernel_spmd
from concourse.masks import make_identity

F32 = mybir.dt.float32
F16 = mybir.dt.float16

NB = 13          # bands
HH = WW = 256    # image
DE = 1024        # embed dim
DO = 128         # out channels
NCORES = 8

WPAD = WW + 2    # 258: zero column at each end
HPADR = HH + 2   # DRAM row padding (zero row top/bottom)
RA = 192         # rows [0, RA) mode A, [RA, 256) mode B
GRP = 8          # psum banks in flight
OSTROWS = 8      # output rows per staging tile / output DMA
ILS = 32         # processed-pair position where B-pair interleaving starts


def _pair_schedule():
    """Processed order of the 128 two-row pairs: A pairs 0..ILS-1 first,
    then (A, A, B) interleave."""
    apairs = list(range(RA // 2))
    bpairs = list(range(RA // 2, HH // 2))
    order = apairs[:ILS]
    ai, bi = ILS, 0
    while ai < len(apairs) or bi < len(bpairs):
        take = min(2, len(apairs) - ai)
        order += apairs[ai:ai + take]
        ai += take
        if bi < len(bpairs):
            order.append(bpairs[bi])
            bi += 1
    return order


def _build_bass(repeat: int = 1, ablate: str = ""):
    ab = set(ablate.split(",")) if ablate else set()
    nc = bacc.Bacc(target_bir_lowering=False, debug=False)

    # host-baked shifted image: planes+ones+replicas (see module docstring);
    # row rr in DRAM = SBUF row rr-1 (zero row top/bottom)
    x_ext = nc.declare_dram_parameter("x118", [118, HPADR, WPAD], F16,
                                      isOutput=False)
    # hyp_pack[p, 0:8]=t_emb (k-major), [8:112]=wv (k,n), [112:116]=b1 (m)
    hp_ext = nc.declare_dram_parameter("hyp_pack", [128, 116], F32,
                                       isOutput=False)
    bbt_ext = nc.declare_dram_parameter("bbT", [1, DO], F16, isOutput=False)
    # w1pm[p, m, k, s] = w1[128k+p, 128m+s]
    w1_ext = nc.declare_dram_parameter("w1pm", [128, 4, 8, 128], F16,
                                       isOutput=False)
    # w2x[s, dx, m, 128dy+d] = w2[128m+s, 9d+3dy+dx]
    w2_ext = nc.declare_dram_parameter("w2x", [128, 3, 4, 384], F16,
                                       isOutput=False)
    b2_ext = nc.declare_dram_parameter("b2x", [3, 384], F16, isOutput=False)
    wb_ext = nc.declare_dram_parameter("wbp", [128, 8, DO], F16, isOutput=False)
    out_ext = nc.declare_dram_parameter("out", [DO, HH, WW], F16, isOutput=True)

    with tile.TileContext(nc) as tc:
        with (
            tc.tile_pool(name="const", bufs=1) as const_pool,
            tc.tile_pool(name="resident", bufs=1) as res_pool,
            tc.tile_pool(name="hyp", bufs=1) as hyp_pool,
        ):
            ident16 = const_pool.tile([128, 128], F16)
            make_identity(nc, ident16[:])

            # ---- input DMAs (all on the SP ring), earliest-needed first ----
            w1p_t = hyp_pool.tile([128, 4, 8, 128], F16)
            nc.sync.dma_start(w1p_t[:, 0], w1_ext.ap()[:, 0])
            b2xT = hyp_pool.tile([1, 3, 384], F16)
            nc.sync.dma_start(
                b2xT[:], b2_ext.ap().rearrange("(o dx) c -> o dx c", o=1)
            )
            hp_t = hyp_pool.tile([128, 116], F32)
            nc.sync.dma_start(hp_t[:], hp_ext.ap())
            bbT_t = hyp_pool.tile([1, DO], F16)
            nc.sync.dma_start(bbT_t[:], bbt_ext.ap())
            for m in range(1, 4):
                nc.sync.dma_start(w1p_t[:, m], w1_ext.ap()[:, m])
            w2x_t = hyp_pool.tile([128, 3, 4, 384], F16)
            nc.sync.dma_start(w2x_t[:, 1], w2_ext.ap()[:, 1])  # dx=1 first

            x118 = res_pool.tile([118, HH, WPAD], F16)
            nc.sync.dma_start(
                x118[0:40, 0:24, :], x_ext.ap()[0:40, 1:25, :]
            )
            wbp_t = hyp_pool.tile([128, 8, DO], F16)
            nc.sync.dma_start(wbp_t[:], wb_ext.ap())
            nc.sync.dma_start(w2x_t[:, 0], w2_ext.ap()[:, 0])
            nc.sync.dma_start(w2x_t[:, 2], w2_ext.ap()[:, 2])
            nc.sync.dma_start(
                x118[0:40, 24:48, :], x_ext.ap()[0:40, 25:49, :]
            )
            # bulk image rows in small pieces: paces the DMA FIFO so tiny
            # latency-critical transfers (lhsT, early outs) never sit behind
            # megabytes of queued bulk
            # piece order follows the interleaved pair schedule: mode-B pairs
            # (rows >= RA, all 118 partitions) start at processed position
            # ~ILS, so the tail plane rows AND the replicas must land before
            # the middle mode-A rows, in small pieces that never let queued
            # bulk starve a consumer
            def piece(p0, p1, lo, hi):
                nc.sync.dma_start(
                    x118[p0:p1, lo:hi, :], x_ext.ap()[p0:p1, lo + 1:hi + 1, :]
                )

            for lo in range(48, 96, 24):
                piece(0, 40, lo, lo + 24)
            for q in range((HH - RA) // 16):
                lo = RA + 16 * q
                piece(0, 40, lo, lo + 16)
                if "norep" not in ab:
                    piece(40, 79, lo, lo + 16)
                    piece(79, 118, lo, lo + 16)
            for lo in range(96, RA, 24):
                piece(0, 40, lo, lo + 24)

            # ---------------- hypernetwork (fp16 in / fp32 psum) ------------
            # PE warm-up filler while the first weight DMAs land: keeps the
            # p-state ramp climbing from t~0.5
            with tc.tile_pool(name="wpsum", bufs=1, space="PSUM") as wpsum:
                ps_w2 = wpsum.tile([128, 128], F16, tag="warm2", bufs=1)
                for _ in range(56):
                    nc.tensor.transpose(ps_w2[:], ident16[:], ident16[:])

            wvv = hp_t[:, 8:112].rearrange("p (k n) -> p k n", k=8)
            ctxT = hyp_pool.tile([128, 8, NB], F16)
            for k in range(8):
                nc.vector.tensor_scalar_add(
                    ctxT[:, k, :], wvv[:, k, :], hp_t[:, k:k + 1]
                )
            ones1 = const_pool.tile([1, NB], F16)
            nc.vector.memset(ones1[:], 1.0)

            lhsT118 = hyp_pool.tile([118, DO], F16)
            lhsT_dx0 = hyp_pool.tile([39, DO], F16)
            lhsT_dx2b = hyp_pool.tile([40, DO], F16)

            with tc.tile_pool(name="hpsum", bufs=2, space="PSUM") as hpsum:
                # hidT[s, m, n] = silu(sum_e w1[e, 128m+s] ctx[e, n] + b1)
                hidT = hyp_pool.tile([128, 4, NB], F16)
                for m in range(4):
                    ps = hpsum.tile([128, NB], F32, tag="hid")
                    for k in range(8):
                        nc.tensor.matmul(
                            ps[:], w1p_t[:, m, k, :], ctxT[:, k, :],
                            start=(k == 0), stop=(k == 7),
                        )
                    nc.scalar.activation(
                        hidT[:, m, :], ps[:],
                        mybir.ActivationFunctionType.Silu,
                        bias=hp_t[:, 112 + m:113 + m],
                    )

                # generated filters by dx chunk (dx=1 gates the conv start);
                # dx=1 scatter on the ACT ring, the rest via Pool SWDGE
                for dxi in (1, 0, 2):
                    ps = hpsum.tile([NB, 3, DO], F32, tag="wgen",
                                    name=f"wg{dxi}")
                    for m in range(4):
                        nc.tensor.matmul(
                            ps[:], hidT[:, m, :], w2x_t[:, dxi, m, :],
                            start=(m == 0), stop=False,
                        )
                    nc.tensor.matmul(
                        ps[:], ones1[:], b2xT[:, dxi, :],
                        start=False, stop=True,
                    )
                    wsb = hyp_pool.tile([NB, 3, DO], F16, name=f"wsb{dxi}")
                    nc.scalar.copy(wsb[:], ps[:])
                    dst = (lhsT118, lhsT_dx0, lhsT_dx2b)[
                        0 if dxi == 1 else (1 if dxi == 0 else 2)
                    ]
                    for dy in range(3):
                        eng = nc.gpsimd if dy == 1 else nc.scalar
                        eng.dma_start(
                            dst[13 * dy:13 * dy + 13, :], wsb[:, dy, :]
                        )

                # bias row, already transposed: psum[1, d] =
                # sum_e s[e] wb[e, d] + (13 bb)[d]
                sT32 = hyp_pool.tile([128, 8, 1], F32)
                nc.vector.reduce_sum(sT32[:], ctxT[:], axis=mybir.AxisListType.X)
                sT = hyp_pool.tile([128, 8, 1], F16)
                nc.vector.tensor_copy(sT[:], sT32[:])
                ps_bt = hpsum.tile([1, DO], F32, tag="bias", bufs=1)
                for k in range(8):
                    nc.tensor.matmul(
                        ps_bt[:], sT[:, k, :], wbp_t[:, k, :],
                        start=(k == 0), stop=False,
                    )
                nc.tensor.matmul(
                    ps_bt[:], ones1[:, 0:1], bbT_t[:],
                    start=False, stop=True,
                )
                bias_row = hyp_pool.tile([1, DO], F16)
                nc.scalar.copy(bias_row[:], ps_bt[:])
                nc.gpsimd.dma_start(lhsT118[39:40, :], bias_row[:])
                nc.gpsimd.dma_start(lhsT_dx2b[39:40, :], bias_row[:])

                # keep the PE p-state ramp alive while the conv gates
                # (w2/wbp transfers) resolve: dependent busy-work
                ps_warm = hpsum.tile([NB, 128], F16, tag="warm", bufs=1)
                for w in range(24):
                    nc.tensor.transpose(
                        ps_warm[:], ctxT[:, w % 8, :], ident16[:]
                    )

            # full 118-row stationary for mode B
            nc.scalar.dma_start(lhsT118[40:79, :], lhsT_dx0[:])
            nc.scalar.dma_start(lhsT118[79:118, :], lhsT_dx2b[0:39, :])

            # ---------------- main loop: dynamic conv -----------------------
            order = _pair_schedule()
            if "norep" in ab:
                order = list(range(HH // 2))
            with (
                tc.tile_pool(name="ostage", bufs=8) as ostage_pool,
                tc.tile_pool(name="ostageb", bufs=3) as ostageb_pool,
                tc.tile_pool(name="cpsum", bufs=GRP, space="PSUM") as cpsum_pool,
            ):
                for _rep in range(repeat):
                    osts = {"A": None, "B": None}
                    cnt = {"A": 0, "B": 0}
                    n_out = 0
                    for i, p in enumerate(order):
                        r0 = 2 * p
                        ps = cpsum_pool.tile(
                            [DO, 2, WW], F32, tag="cps", name=f"cps{i % GRP}"
                        )
                        if r0 < RA or "norep" in ab:
                            stream = "A"
                            nc.tensor.matmul(
                                ps[:], lhsT118[0:39, :],
                                x118[0:39, r0:r0 + 2, 1:1 + WW],
                                start=True, stop=False,
                            )
                            if "mm1" not in ab:
                                nc.tensor.matmul(
                                    ps[:], lhsT_dx0[:],
                                    x118[0:39, r0:r0 + 2, 0:WW],
                                    start=False, stop=False,
                                )
                                nc.tensor.matmul(
                                    ps[:], lhsT_dx2b[:],
                                    x118[0:40, r0:r0 + 2, 2:2 + WW],
                                    start=False, stop=True,
                                )
                        else:
                            stream = "B"
                            nc.tensor.matmul(
                                ps[:], lhsT118[:],
                                x118[:, r0:r0 + 2, 1:1 + WW],
                                start=True, stop=True,
                            )
                        c = cnt[stream]
                        if c % 4 == 0:
                            pool = ostage_pool if stream == "A" else ostageb_pool
                            osts[stream] = pool.tile(
                                [DO, OSTROWS, WW], F16, tag="ost" + stream,
                                name="ost" + stream,
                            )
                        ost = osts[stream]
                        e = c % 4
                        if i % 2 == 0:
                            nc.vector.tensor_copy(
                                ost[:, 2 * e:2 * e + 2, :], ps[:]
                            )
                        else:
                            nc.scalar.copy(ost[:, 2 * e:2 * e + 2, :], ps[:])
                        cnt[stream] = c + 1
                        last_ost = (
                            (stream == "A" and c >= RA // 2 - 4)
                            or (stream == "B" and c >= (HH - RA) // 2 - 4)
                        ) and "norep" not in ab
                        if "outslim" in ab:
                            if c % 4 == 3:
                                dma_eng = (nc.gpsimd, nc.sync)[n_out % 2]
                                n_out += 1
                                dma_eng.dma_start(
                                    out_ext.ap()[:, r0 - 6:r0 + 2, 0:16],
                                    ost[:, :, 0:16],
                                )
                        elif last_ost:
                            # final staging tile per stream: drain per pair so
                            # the tail transfer isn't one long serialized DMA
                            dma_eng = (nc.gpsimd, nc.sync)[n_out % 2]
                            n_out += 1
                            dma_eng.dma_start(
                                out_ext.ap()[:, r0:r0 + 2, :],
                                ost[:, 2 * e:2 * e + 2, :],
                            )
                        elif c % 4 == 3:
                            y0 = r0 - 6
                            dma_eng = (nc.gpsimd, nc.sync)[n_out % 2]
                            n_out += 1
                            dma_eng.dma_start(
                                out_ext.ap()[:, y0:y0 + OSTROWS, :], ost[:]
                            )
    if not nc.is_finalized():
        nc.finalize()
    return nc


_NC_CACHE = None


def _get_bass():
    global _NC_CACHE
    if _NC_CACHE is None:
        _NC_CACHE = _build_bass()
    return _NC_CACHE


def _prep_in_maps(inputs):
    xf = np.asarray(inputs["x"], dtype=np.float32).astype(np.float16)
    B = xf.shape[0]
    # padded source: xp[n, rr, c] = x[n, rr-1, c-1], zero border
    xp = np.zeros((B, NB, HPADR, WPAD), np.float16)
    xp[:, :, 1:HH + 1, 1:WW + 1] = xf
    x118 = np.zeros((B, 118, HPADR, WPAD), np.float16)
    for dy in range(3):
        pl = np.zeros((B, NB, HPADR, WPAD), np.float16)
        if dy == 0:
            pl[:, :, 1:] = xp[:, :, :-1]
        elif dy == 1:
            pl[:, :] = xp
        else:
            pl[:, :, :-1] = xp[:, :, 1:]
        x118[:, 13 * dy:13 * dy + 13] = pl
        # dx replicas: dx0[c] = plane[c-1], dx2[c] = plane[c+1]
        x118[:, 40 + 13 * dy:53 + 13 * dy, :, 1:] = pl[:, :, :, :-1]
        x118[:, 79 + 13 * dy:92 + 13 * dy, :, :-1] = pl[:, :, :, 1:]
    x118[:, 39] = 1.0

    t_emb = np.asarray(inputs["t_emb"], dtype=np.float32)
    wv = np.asarray(inputs["wv_embs"], dtype=np.float32)
    w1 = np.asarray(inputs["w1"], dtype=np.float32)
    b1 = np.asarray(inputs["b1"], dtype=np.float32)
    w2 = np.asarray(inputs["w2"], dtype=np.float32)
    b2 = np.asarray(inputs["b2"], dtype=np.float32)
    wb = np.asarray(inputs["wb"], dtype=np.float32)
    bb = np.asarray(inputs["bb"], dtype=np.float32)

    # hyp_pack: [0:8] t (k-major), [8:112] wv (k, n), [112:116] b1
    hp = np.zeros((B, 128, 116), np.float32)
    hp[:, :, 0:8] = t_emb.reshape(B, 8, 128).transpose(0, 2, 1)
    hp[:, :, 8:112] = wv.reshape(B, NB, 8, 128).transpose(0, 3, 2, 1).reshape(
        B, 128, 104
    )
    hp[:, :, 112:116] = b1.reshape(4, 128).T[None]
    bbT = np.ascontiguousarray((NB * bb).reshape(1, DO)).astype(np.float16)

    w1pm = np.ascontiguousarray(
        w1.reshape(8, 128, 4, 128).transpose(1, 2, 0, 3)
    ).astype(np.float16)
    w2x = np.ascontiguousarray(
        w2.reshape(4, 128, DO, 3, 3).transpose(1, 4, 0, 3, 2).reshape(
            128, 3, 4, 384
        )
    ).astype(np.float16)
    b2x = np.ascontiguousarray(
        b2.reshape(DO, 3, 3).transpose(2, 1, 0).reshape(3, 384)
    ).astype(np.float16)
    wbp = np.ascontiguousarray(
        wb.reshape(8, 128, DO).transpose(1, 0, 2)
    ).astype(np.float16)

    return [
        {
            "x118": x118[b], "hyp_pack": np.ascontiguousarray(hp[b]),
            "bbT": bbT, "w1pm": w1pm, "w2x": w2x, "b2x": b2x, "wbp": wbp,
        }
        for b in range(NCORES)
    ]


def kernel(**inputs) -> np.ndarray:
    nc = _get_bass()
    in_maps = _prep_in_maps(inputs)
    res = run_bass_kernel_spmd(nc, in_maps, list(range(NCORES)))
    return np.stack(
        [res.results[b]["out"] for b in range(NCORES)], axis=0
    ).astype(np.float32)


if __name__ == "__main__":
    rng = np.random.default_rng(0)
    demo = {
        "x": rng.standard_normal((NCORES, NB, HH, WW), dtype=np.float32),
        "t_emb": rng.standard_normal((NCORES, DE), dtype=np.float32),
        "wv_embs": rng.standard_normal((NCORES, NB, DE), dtype=np.float32),
        "w1": rng.standard_normal((DE, 4 * DO), dtype=np.float32) * 0.02,
        "b1": np.zeros(4 * DO, np.float32),
        "w2": rng.standard_normal((4 * DO, DO * 9), dtype=np.float32) * 0.02,
        "b2": np.zeros(DO * 9, np.float32),
        "wb": rng.standard_normal((DE, DO), dtype=np.float32) * 0.02,
        "bb": np.zeros(DO, np.float32),
    }
    out = kernel(**demo)
    print("out", out.shape, out.dtype, float(np.abs(out).mean()))


# revision 28
# speedup vs baseline: 1.5552x; 1.1160x over previous
"""Trainium2 Bass kernel for nn_DiffusionDynamicInput.

Reference computation (per sample b):
    ctx  = wv_embs[b] + t_emb[b]                       (13, 1024)
    hid  = silu(ctx @ w1 + b1)                         (13, 512)
    wgen = (hid @ w2 + b2).reshape(13, 128, 9)         per-(band) 3x3 filters
    out[d,h,w] = sum_{n,dy,dx} wgen[n,d,(dy,dx)] * x[b,n,h+dy,w+dx]   (SAME)
    bias = (ctx @ wb + bb).sum(axis=0)                 (128,)
    out += bias[:, None, None]

Sharding: data-parallel over B=8 across the 8 NeuronCores (one sample per
core).

Dynamic conv: an fp16 SBUF tile x118 [118, 256, 258] holds
  partitions  0..38  dy-shifted planes of the image (dy-major, host-baked)
  partition   39     all-ones (the generated bias rides the matmuls as an
                     extra contraction row against this partition)
  partitions 40..117 dx-shifted replicas of the planes, rows >= RA only
Rows < RA run "mode A": three PSUM-accumulated matmuls (contraction
39/39/40) with the dx shift as a free-dim column offset. Rows >= RA run
"mode B": one matmul with contraction 118 over (dx, dy, n) + bias. RA
balances PE column-streaming against the ~360 GB/s DMA budget (the fp16
output write dominates). Mode-B pairs are interleaved among mode-A pairs
(pattern A,A,B) so PSUM-eviction load stays smooth.

PSUM eviction is a plain fp32->fp16 copy alternating DVE/ACT; output DMAs
issue only from the SP/Pool queues so no sequencer mixes evictions with
blocking DMA waits. Output is written fp16 (rel tol 2e-2 vs ~4e-4
achieved) and upcast on host. Hypernetwork weights arrive host-permuted
(w1 m-major, w2 dx-major) so the dx=1 filter block - which gates the conv
start - is generated as early as possible.
"""

import numpy as np

import concourse.bacc as bacc
import concourse.mybir as mybir
import concourse.tile as tile
from concourse.bass_utils import run_bass_kernel_spmd
from concourse.masks import make_identity

F32 = mybir.dt.float32
F16 = mybir.dt.float16

NB = 13          # bands
HH = WW = 256    # image
DE = 1024        # embed dim
DO = 128         # out channels
NCORES = 8

WPAD = WW + 2    # 258: zero column at each end
HPADR = HH + 2   # DRAM row padding (zero row top/bottom)
RA = 192         # rows [0, RA) mode A, [RA, 256) mode B
GRP = 8          # psum banks in flight
OSTROWS = 8      # output rows per staging tile / output DMA
ILS = 32         # processed-pair position where B-pair interleaving starts


def _pair_schedule():
    """Processed order of the 128 two-row pairs: A pairs 0..ILS-1 first,
    then (A, A, B) interleave."""
    apairs = list(range(RA // 2))
    bpairs = list(range(RA // 2, HH // 2))
    order = apairs[:ILS]
    ai, bi = ILS, 0
    while ai < len(apairs) or bi < len(bpairs):
        take = min(2, len(apairs) - ai)
        order += apairs[ai:ai + take]
        ai += take
        if bi < len(bpairs):
            order.append(bpairs[bi])
            bi += 1
    return order


def _build_bass(repeat: int = 1, ablate: str = ""):
    ab = set(ablate.split(",")) if ablate else set()
    nc = bacc.Bacc(target_bir_lowering=False, debug=False)

    # host-baked shifted image: planes+ones+replicas (see module docstring);
    # row rr in DRAM = SBUF row rr-1 (zero row top/bottom)
    x_ext = nc.declare_dram_parameter("x118", [118, HPADR, WPAD], F16,
                                      isOutput=False)
    # hyp_pack[p, 0:8]=t_emb (k-major), [8:112]=wv (k,n), [112:116]=b1 (m)
    hp_ext = nc.declare_dram_parameter("hyp_pack", [128, 116], F32,
                                       isOutput=False)
    bbt_ext = nc.declare_dram_parameter("bbT", [1, DO], F16, isOutput=False)
    # w1pm[p, m, k, s] = w1[128k+p, 128m+s]
    w1_ext = nc.declare_dram_parameter("w1pm", [128, 4, 8, 128], F16,
                                       isOutput=False)
    # w2x[s, dx, m, 128dy+d] = w2[128m+s, 9d+3dy+dx]
    w2_ext = nc.declare_dram_parameter("w2x", [128, 3, 4, 384], F16,
                                       isOutput=False)
    b2_ext = nc.declare_dram_parameter("b2x", [3, 384], F16, isOutput=False)
    wb_ext = nc.declare_dram_parameter("wbp", [128, 8, DO], F16, isOutput=False)
    out_ext = nc.declare_dram_parameter("out", [DO, HH, WW], F16, isOutput=True)

    with tile.TileContext(nc) as tc:
        with (
            tc.tile_pool(name="const", bufs=1) as const_pool,
            tc.tile_pool(name="resident", bufs=1) as res_pool,
            tc.tile_pool(name="hyp", bufs=1) as hyp_pool,
        ):
            ident16 = const_pool.tile([128, 128], F16)
            make_identity(nc, ident16[:])

            # ---- input DMAs (all on the SP ring), earliest-needed first ----
            w1p_t = hyp_pool.tile([128, 4, 8, 128], F16)
            nc.sync.dma_start(w1p_t[:, 0], w1_ext.ap()[:, 0])
            b2xT = hyp_pool.tile([1, 3, 384], F16)
            nc.sync.dma_start(
                b2xT[:], b2_ext.ap().rearrange("(o dx) c -> o dx c", o=1)
            )
            hp_t = hyp_pool.tile([128, 116], F32)
            nc.sync.dma_start(hp_t[:], hp_ext.ap())
            bbT_t = hyp_pool.tile([1, DO], F16)
            nc.sync.dma_start(bbT_t[:], bbt_ext.ap())
            for m in range(1, 4):
                nc.sync.dma_start(w1p_t[:, m], w1_ext.ap()[:, m])
            w2x_t = hyp_pool.tile([128, 3, 4, 384], F16)
            nc.sync.dma_start(w2x_t[:, 1], w2_ext.ap()[:, 1])  # dx=1 first

            x118 = res_pool.tile([118, HH, WPAD], F16)
            nc.sync.dma_start(
                x118[0:40, 0:24, :], x_ext.ap()[0:40, 1:25, :]
            )
            wbp_t = hyp_pool.tile([128, 8, DO], F16)
            nc.sync.dma_start(wbp_t[:], wb_ext.ap())
            nc.sync.dma_start(w2x_t[:, 0], w2_ext.ap()[:, 0])
            nc.sync.dma_start(w2x_t[:, 2], w2_ext.ap()[:, 2])
            nc.sync.dma_start(
                x118[0:40, 24:48, :], x_ext.ap()[0:40, 25:49, :]
            )
            # bulk image rows in small pieces: paces the DMA FIFO so tiny
            # latency-critical transfers (lhsT, early outs) never sit behind
            # megabytes of queued bulk
            # piece order follows the interleaved pair schedule: mode-B pairs
            # (rows >= RA, all 118 partitions) start at processed position
            # ~ILS, so the tail plane rows AND the replicas must land before
            # the middle mode-A rows, in small pieces that never let queued
            # bulk starve a consumer
            def piece(p0, p1, lo, hi):
                nc.sync.dma_start(
                    x118[p0:p1, lo:hi, :], x_ext.ap()[p0:p1, lo + 1:hi + 1, :]
                )

            for lo in range(48, 96, 24):
                piece(0, 40, lo, lo + 24)
            for q in range((HH - RA) // 16):
                lo = RA + 16 * q
                piece(0, 40, lo, lo + 16)
                if "norep" not in ab:
                    piece(40, 79, lo, lo + 16)
                    piece(79, 118, lo, lo + 16)
            for lo in range(96, RA, 24):
                piece(0, 40, lo, lo + 24)

            # ---------------- hypernetwork (fp16 in / fp32 psum) ------------
            # PE warm-up filler while the first weight DMAs land: keeps the
            # p-state ramp climbing from t~0.5
            with tc.tile_pool(name="wpsum", bufs=1, space="PSUM") as wpsum:
                ps_w2 = wpsum.tile([128, 128], F16, tag="warm2", bufs=1)
                for _ in range(56):
                    nc.tensor.transpose(ps_w2[:], ident16[:], ident16[:])

            wvv = hp_t[:, 8:112].rearrange("p (k n) -> p k n", k=8)
            ctxT = hyp_pool.tile([128, 8, NB], F16)
            for k in range(8):
                nc.vector.tensor_scalar_add(
                    ctxT[:, k, :], wvv[:, k, :], hp_t[:, k:k + 1]
                )
            ones1 = const_pool.tile([1, NB], F16)
            nc.vector.memset(ones1[:], 1.0)

            lhsT118 = hyp_pool.tile([118, DO], F16)
            lhsT_dx0 = hyp_pool.tile([39, DO], F16)
            lhsT_dx2b = hyp_pool.tile([40, DO], F16)

            with tc.tile_pool(name="hpsum", bufs=2, space="PSUM") as hpsum:
                # hidT[s, m, n] = silu(sum_e w1[e, 128m+s] ctx[e, n] + b1)
                hidT = hyp_pool.tile([128, 4, NB], F16)
                for m in range(4):
                    ps = hpsum.tile([128, NB], F32, tag="hid")
                    for k in range(8):
                        nc.tensor.matmul(
                            ps[:], w1p_t[:, m, k, :], ctxT[:, k, :],
                            start=(k == 0), stop=(k == 7),
                        )
                    nc.scalar.activation(
                        hidT[:, m, :], ps[:],
                        mybir.ActivationFunctionType.Silu,
                        bias=hp_t[:, 112 + m:113 + m],
                    )

                # generated filters by dx chunk (dx=1 gates the conv start);
                # dx=1 scatter on the ACT ring, the rest via Pool SWDGE
                for dxi in (1, 0, 2):
                    ps = hpsum.tile([NB, 3, DO], F32, tag="wgen",
                                    name=f"wg{dxi}")
                    for m in range(4):
                        nc.tensor.matmul(
                            ps[:], hidT[:, m, :], w2x_t[:, dxi, m, :],
                            start=(m == 0), stop=False,
                        )
                    nc.tensor.matmul(
                        ps[:], ones1[:], b2xT[:, dxi, :],
                        start=False, stop=True,
                    )
                    wsb = hyp_pool.tile([NB, 3, DO], F16, name=f"wsb{dxi}")
                    nc.scalar.copy(wsb[:], ps[:])
                    dst = (lhsT118, lhsT_dx0, lhsT_dx2b)[
                        0 if dxi == 1 else (1 if dxi == 0 else 2)
                    ]
                    for dy in range(3):
                        eng = nc.gpsimd if dy == 1 else nc.scalar
                        eng.dma_start(
                            dst[13 * dy:13 * dy + 13, :], wsb[:, dy, :]
                        )

                # bias row, already transposed: psum[1, d] =
                # sum_e s[e] wb[e, d] + (13 bb)[d]
                sT32 = hyp_pool.tile([128, 8, 1], F32)
                nc.vector.reduce_sum(sT32[:], ctxT[:], axis=mybir.AxisListType.X)
                sT = hyp_pool.tile([128, 8, 1], F16)
                nc.vector.tensor_copy(sT[:], sT32[:])
                ps_bt = hpsum.tile([1, DO], F32, tag="bias", bufs=1)
                for k in range(8):
                    nc.tensor.matmul(
                        ps_bt[:], sT[:, k, :], wbp_t[:, k, :],
                        start=(k == 0), stop=False,
                    )
                nc.tensor.matmul(
                    ps_bt[:], ones1[:, 0:1], bbT_t[:],
                    start=False, stop=True,
                )
                bias_row = hyp_pool.tile([1, DO], F16)
                nc.scalar.copy(bias_row[:], ps_bt[:])
                nc.gpsimd.dma_start(lhsT118[39:40, :], bias_row[:])
                nc.gpsimd.dma_start(lhsT_dx2b[39:40, :], bias_row[:])

                # keep the PE p-state ramp alive while the conv gates
                # (w2/wbp transfers) resolve: dependent busy-work
                ps_warm = hpsum.tile([NB, 128], F16, tag="warm", bufs=1)
                for w in range(24):
                    nc.tensor.transpose(
                        ps_warm[:], ctxT[:, w % 8, :], ident16[:]
                    )

            # full 118-row stationary for mode B
            nc.gpsimd.dma_start(lhsT118[40:79, :], lhsT_dx0[:])
            nc.gpsimd.dma_start(lhsT118[79:118, :], lhsT_dx2b[0:39, :])

            # ---------------- main loop: dynamic conv -----------------------
            order = _pair_schedule()
            if "norep" in ab:
                order = list(range(HH // 2))
            with (
                tc.tile_pool(name="ostage", bufs=8) as ostage_pool,
                tc.tile_pool(name="ostageb", bufs=3) as ostageb_pool,
                tc.tile_pool(name="cpsum", bufs=GRP, space="PSUM") as cpsum_pool,
            ):
                for _rep in range(repeat):
                    osts = {"A": None, "B": None}
                    cnt = {"A": 0, "B": 0}
                    n_out = 0
                    for i, p in enumerate(order):
                        r0 = 2 * p
                        ps = cpsum_pool.tile(
                            [DO, 2, WW], F32, tag="cps", name=f"cps{i % GRP}"
                        )
                        if r0 < RA or "norep" in ab:
                            stream = "A"
                            nc.tensor.matmul(
                                ps[:], lhsT118[0:39, :],
                                x118[0:39, r0:r0 + 2, 1:1 + WW],
                                start=True, stop=False,
                            )
                            if "mm1" not in ab:
                                nc.tensor.matmul(
                                    ps[:], lhsT_dx0[:],
                                    x118[0:39, r0:r0 + 2, 0:WW],
                                    start=False, stop=False,
                                )
                                nc.tensor.matmul(
                                    ps[:], lhsT_dx2b[:],
                                    x118[0:40, r0:r0 + 2, 2:2 + WW],
                                    start=False, stop=True,
                                )
                        else:
                            stream = "B"
                            nc.tensor.matmul(
                                ps[:], lhsT118[:],
                                x118[:, r0:r0 + 2, 1:1 + WW],
                                start=True, stop=True,
                            )
                        c = cnt[stream]
                        if c % 4 == 0:
                            pool = ostage_pool if stream == "A" else ostageb_pool
                            osts[stream] = pool.tile(
                                [DO, OSTROWS, WW], F16, tag="ost" + stream,
                                name="ost" + stream,
                            )
                        ost = osts[stream]
                        e = c % 4
                        # DVE handles all early evictions while the ACT
                        # ring drains its lhsT-scatter backlog
                        if i < 12 or i % 2 == 0:
                            nc.vector.tensor_copy(
                                ost[:, 2 * e:2 * e + 2, :], ps[:]
                            )
                        else:
                            nc.scalar.copy(ost[:, 2 * e:2 * e + 2, :], ps[:])
                        cnt[stream] = c + 1
                        last_ost = (
                            (stream == "A" and c >= RA // 2 - 4)
                            or (stream == "B" and c >= (HH - RA) // 2 - 4)
                        ) and "norep" not in ab
                        if "outslim" in ab:
                            if c % 4 == 3:
                                dma_eng = (nc.gpsimd, nc.sync)[n_out % 2]
                                n_out += 1
                                dma_eng.dma_start(
                                    out_ext.ap()[:, r0 - 6:r0 + 2, 0:16],
                                    ost[:, :, 0:16],
                                )
                        elif last_ost:
                            # final staging tile per stream: drain per pair so
                            # the tail transfer isn't one long serialized DMA
                            dma_eng = (nc.gpsimd, nc.sync)[n_out % 2]
                            n_out += 1
                            dma_eng.dma_start(
                                out_ext.ap()[:, r0:r0 + 2, :],
                                ost[:, 2 * e:2 * e + 2, :],
                            )
                        elif c % 4 == 3:
                            y0 = r0 - 6
                            dma_eng = (nc.gpsimd, nc.sync)[n_out % 2]
                            n_out += 1
                            dma_eng.dma_start(
                                out_ext.ap()[:, y0:y0 + OSTROWS, :], ost[:]
                            )
    if not nc.is_finalized():
        nc.finalize()
    return nc


_NC_CACHE = None


def _get_bass():
    global _NC_CACHE
    if _NC_CACHE is None:
        _NC_CACHE = _build_bass()
    return _NC_CACHE


def _prep_in_maps(inputs):
    xf = np.asarray(inputs["x"], dtype=np.float32).astype(np.float16)
    B = xf.shape[0]
    # padded source: xp[n, rr, c] = x[n, rr-1, c-1], zero border
    xp = np.zeros((B, NB, HPADR, WPAD), np.float16)
    xp[:, :, 1:HH + 1, 1:WW + 1] = xf
    x118 = np.zeros((B, 118, HPADR, WPAD), np.float16)
    for dy in range(3):
        pl = np.zeros((B, NB, HPADR, WPAD), np.float16)
        if dy == 0:
            pl[:, :, 1:] = xp[:, :, :-1]
        elif dy == 1:
            pl[:, :] = xp
        else:
            pl[:, :, :-1] = xp[:, :, 1:]
        x118[:, 13 * dy:13 * dy + 13] = pl
        # dx replicas: dx0[c] = plane[c-1], dx2[c] = plane[c+1]
        x118[:, 40 + 13 * dy:53 + 13 * dy, :, 1:] = pl[:, :, :, :-1]
        x118[:, 79 + 13 * dy:92 + 13 * dy, :, :-1] = pl[:, :, :, 1:]
    x118[:, 39] = 1.0

    t_emb = np.asarray(inputs["t_emb"], dtype=np.float32)
    wv = np.asarray(inputs["wv_embs"], dtype=np.float32)
    w1 = np.asarray(inputs["w1"], dtype=np.float32)
    b1 = np.asarray(inputs["b1"], dtype=np.float32)
    w2 = np.asarray(inputs["w2"], dtype=np.float32)
    b2 = np.asarray(inputs["b2"], dtype=np.float32)
    wb = np.asarray(inputs["wb"], dtype=np.float32)
    bb = np.asarray(inputs["bb"], dtype=np.float32)

    # hyp_pack: [0:8] t (k-major), [8:112] wv (k, n), [112:116] b1
    hp = np.zeros((B, 128, 116), np.float32)
    hp[:, :, 0:8] = t_emb.reshape(B, 8, 128).transpose(0, 2, 1)
    hp[:, :, 8:112] = wv.reshape(B, NB, 8, 128).transpose(0, 3, 2, 1).reshape(
        B, 128, 104
    )
    hp[:, :, 112:116] = b1.reshape(4, 128).T[None]
    bbT = np.ascontiguousarray((NB * bb).reshape(1, DO)).astype(np.float16)

    w1pm = np.ascontiguousarray(
        w1.reshape(8, 128, 4, 128).transpose(1, 2, 0, 3)
    ).astype(np.float16)
    w2x = np.ascontiguousarray(
        w2.reshape(4, 128, DO, 3, 3).transpose(1, 4, 0, 3, 2).reshape(
            128, 3, 4, 384
        )
    ).astype(np.float16)
    b2x = np.ascontiguousarray(
        b2.reshape(DO, 3, 3).transpose(2, 1, 0).reshape(3, 384)
    ).astype(np.float16)
    wbp = np.ascontiguousarray(
        wb.reshape(8, 128, DO).transpose(1, 0, 2)
    ).astype(np.float16)

    return [
        {
            "x118": x118[b], "hyp_pack": np.ascontiguousarray(hp[b]),
            "bbT": bbT, "w1pm": w1pm, "w2x": w2x, "b2x": b2x, "wbp": wbp,
        }
        for b in range(NCORES)
    ]


def kernel(**inputs) -> np.ndarray:
    nc = _get_bass()
    in_maps = _prep_in_maps(inputs)
    res = run_bass_kernel_spmd(nc, in_maps, list(range(NCORES)))
    return np.stack(
        [res.results[b]["out"] for b in range(NCORES)], axis=0
    ).astype(np.float32)


if __name__ == "__main__":
    rng = np.random.default_rng(0)
    demo = {
        "x": rng.standard_normal((NCORES, NB, HH, WW), dtype=np.float32),
        "t_emb": rng.standard_normal((NCORES, DE), dtype=np.float32),
        "wv_embs": rng.standard_normal((NCORES, NB, DE), dtype=np.float32),
        "w1": rng.standard_normal((DE, 4 * DO), dtype=np.float32) * 0.02,
        "b1": np.zeros(4 * DO, np.float32),
        "w2": rng.standard_normal((4 * DO, DO * 9), dtype=np.float32) * 0.02,
        "b2": np.zeros(DO * 9, np.float32),
        "wb": rng.standard_normal((DE, DO), dtype=np.float32) * 0.02,
        "bb": np.zeros(DO, np.float32),
    }
    out = kernel(**demo)
    print("out", out.shape, out.dtype, float(np.abs(out).mean()))
